# revision 23
# baseline (speedup 1.0000x reference)
"""Distributed multi-head attention (RoPE, non-causal) for 8 TRN2 NeuronCores.

Problem: B=2, S=2048, DIM=768, H=12, HEAD_DIM=64, f32 I/O.

Sharding: 24 (batch, head) pairs -> core c handles batch c//4 and heads
3*(c%4) .. 3*(c%4)+2.  Per core (bf16 matmuls, f32 PSUM):
  * QKV projection emitted K-first (k0,k1 over all S, then q01 block 0) so
    the first exp fires early; remaining projection work (q2k2, v, q01
    slabs 1-3) is emitted inside block 0's attention stream as PE filler.
  * RoPE fused out of PSUM (deinterleaved channels; rotate_half = 32-row
    partition swap via DMA, sign folded into the sin table).
  * scores: heads 0,1 run on PE row-halves (tile_position (0,0)/(64,0),
    M=128) straight from the qkv m-blocks -- no duplication DMAs; head 2
    pairs even/odd key chunks via 2 partition-shift copies.  exp on
    ScalarE from 2-bank PSUM tiles (scale=1/8 folded in; scores*scale is
    bounded ~1 so no max-subtraction is needed).
  * out^T accumulated via lhsT=[v | ones | pad] (M=128); the softmax
    denominator lands on psum partition 64; per head: DVE copy to SBUF +
    reciprocal_approx_fast (the custom-DVE reciprocal must NOT read PSUM
    directly -- that was a silent-garbage bug), K=1 ones-matmul broadcast,
    one DVE multiply.
  * One AllGather per 512-query block ([192,512] -> [768,512]), issued as
    soon as that block's heads finish.  Each core projects only its own
    512-row slice: per-block cond-predicated DMAs (ap_or_oob) keep the
    SPMD program identical on all cores while depending only on this
    core's gather.  Bias enters via a precomputed [128,768] broadcast
    added during the PSUM->SBUF copy.
Host side only shards/permutes/casts inputs and concatenates the 8
output slices.
"""

import sys

sys.path.insert(0, "/opt/trn_rl_repo")

import numpy as np
import ml_dtypes

import concourse.bass as bass
import concourse.mybir as mybir
import concourse.tile as tile
from concourse import bacc, bass_utils

BF16 = mybir.dt.bfloat16
F32 = mybir.dt.float32
AF = mybir.ActivationFunctionType

B, S, DIM, H, DH = 2, 2048, 768, 12, 64
THETA = 10000.0
N_CORES = 8
GROUPS = [[0, 1, 2, 3], [4, 5, 6, 7]]
HL = 3           # heads per core
SC = S // 4      # per-core output row slice (512)
KC = DIM // 128  # 6 contraction chunks
NJ = S // 128    # 16 key chunks
NB = 4           # query blocks per core batch

_CACHED = {}

# --------------------------------------------------------------------------
# Custom DVE exp(x/8): degree-7 poly, p(0)=1, fitted minimax-relative on
# |x|<=24 (raw scores span +-21.4).  Split into two 7-stage ops:
#   PART1: H = (((c7*x + c6)*x + c5)*x + c4)*x        (c4 spilled via in1)
#   PART2: out = ((((H + c3)*x + c2)*x + c1)*x) + 1
_EXP_C = [
    1.25124039e-01, 7.79104663e-03, 3.19739592e-04, 1.00846613e-05,
    2.82333267e-07, 6.39666992e-09, 7.30440201e-11,
]


def _register_exp_ops():
    if "ops" in _CACHED:
        return _CACHED["ops"]
    import concourse.dve_ops as dve_ops
    from concourse.dve_ops import DveOp
    from concourse.dve_spec import C0, C1, C2, C3, One, Spec, Src0, Src1, \
        _spill_c3_to_src1, lower
    from concourse.dve_uop import DveOpSpec

    def ref1(in0, in1, c0, c1, c2):
        x = in0.astype(np.float32)
        return ((((c0 * x + c1) * x + c2) * x + in1) * x).astype(np.float32)

    def ref2(in0, in1, c0, c1, c2):
        x = in0.astype(np.float32)
        h = in1.astype(np.float32)
        return (((h + c0) * x + c1) * x + c2) * x + np.float32(1.0)

    body1 = _spill_c3_to_src1((((Src0 * C0 + C1) * Src0 + C2) * Src0 + C3) * Src0)
    body2 = (((Src1 + C0) * Src0 + C1) * Src0 + C2) * Src0 + One

    def mk(name, body, ref):
        if name in dve_ops._SUB_OPCODE_FOR_NAME:
            return next(o for o in dve_ops.OPS if o.name == name)
        spec = Spec(body=body, reference=ref)
        shas = {}
        for ver in ("v3", "v4"):
            shas[ver] = DveOpSpec(
                name=name, opcode=31, uops=lower(spec, ver=ver), rd1_en=True
            ).sha(ver)
        op = DveOp(name, spec, subdim=False, uops_sha=shas)
        dve_ops.OPS.append(op)
        dve_ops.CUSTOM_DVE_SPECS[name] = spec
        dve_ops._SUB_OPCODE_FOR_NAME[name] = (
            max(dve_ops._SUB_OPCODE_FOR_NAME.values()) + 1
        )
        assert max(dve_ops._SUB_OPCODE_FOR_NAME.values()) < 0x20
        return op

    e1 = mk("EXP8_PART1_ANT", body1, ref1)
    e2 = mk("EXP8_PART2_ANT", body2, ref2)
    _CACHED["ops"] = (e1, e2)
    return e1, e2




def _build():
    """Build the SPMD Bacc graph (identical on all 8 cores)."""
    EXP1, EXP2 = _register_exp_ops()
    c1, c2, c3, c4, c5, c6, c7 = [float(np.float32(v)) for v in _EXP_C]
    nc = bacc.Bacc(None, target_bir_lowering=False)

    xT = nc.declare_dram_parameter("xT", [DIM, S], BF16, isOutput=False)
    wqk = nc.declare_dram_parameter("wqk", [DIM, 2 * HL * DH], BF16, isOutput=False)
    wv = nc.declare_dram_parameter("wv", [DIM, HL * DH], BF16, isOutput=False)
    cosq = nc.declare_dram_parameter("cosq", [128, S], BF16, isOutput=False)
    sinq = nc.declare_dram_parameter("sinq", [128, S], BF16, isOutput=False)
    wp = nc.declare_dram_parameter("wp", [DIM, DIM], BF16, isOutput=False)
    bp = nc.declare_dram_parameter("bp", [1, DIM], F32, isOutput=False)
    flags = nc.declare_dram_parameter("flags", [1, NB], mybir.dt.uint32,
                                      isOutput=False)
    out_d = nc.declare_dram_parameter("out", [SC, DIM], F32, isOutput=True)

    scale = DH ** -0.5

    with tile.TileContext(nc) as tc:
        with (
            tc.tile_pool(name="const", bufs=1) as const,
            tc.tile_pool(name="work", bufs=2) as work,
            tc.tile_pool(name="psum", bufs=2, space="PSUM") as psum,
            tc.tile_pool(name="dram", bufs=1, space="DRAM") as dram,
        ):
            # ---- load inputs (ACT HW-DGE queue, in dependency order) -------
            xT_sb = const.tile([128, KC, S], BF16)
            wqk_sb = const.tile([128, KC, 2 * HL * DH], BF16)
            wv_sb = const.tile([128, KC, HL * DH], BF16)
            wp_sb = const.tile([128, KC, DIM], BF16)
            cos_sb = const.tile([128, S], BF16)
            sin_sb = const.tile([128, S], BF16)
            bp_sb = const.tile([1, DIM], F32)
            for k in range(KC):
                nc.scalar.dma_start(wqk_sb[:, k, :], wqk[k * 128:(k + 1) * 128, :])
                nc.sync.dma_start(xT_sb[:, k, :], xT[k * 128:(k + 1) * 128, :])
            nc.scalar.dma_start(cos_sb[:], cosq[:])
            nc.scalar.dma_start(sin_sb[:], sinq[:])
            nc.scalar.dma_start(bp_sb[:], bp[:])
            for k in range(KC):
                nc.scalar.dma_start(wv_sb[:, k, :], wv[k * 128:(k + 1) * 128, :])
            for k in range(KC):
                nc.scalar.dma_start(wp_sb[:, k, :], wp[k * 128:(k + 1) * 128, :])

            ones128 = const.tile([128, 128], F32)
            nc.vector.memset(ones128[:], 1.0)
            c4b = const.tile([128, 1], F32)
            nc.vector.memset(c4b[:], c4)

            # preload the exp table set while ScalarE is idle
            warm_sb = work.tile([1, 16], F32, tag="warm")
            nc.scalar.activation(warm_sb[:], ones128[0:1, 0:16], AF.Exp)

            # ---- qk^T = wqk.T @ xT with fused RoPE -------------------------
            # wqk column order [q0, q1 | k0, k1 | q2, k2], channels
            # deinterleaved per head so rotate_half is a 32-partition swap.
            QKM = 2 * HL * DH // 128  # 3 M-blocks
            qkb = const.tile([128, QKM, S], BF16)

            def emit_qk_tile(mb, sb, ptag="ps_mm"):
                sl = slice(sb * 512, (sb + 1) * 512)
                ps = psum.tile([128, 512], F32, tag=ptag)
                for k in range(KC):
                    nc.tensor.matmul(
                        ps[:],
                        wqk_sb[:, k, mb * 128:(mb + 1) * 128],
                        xT_sb[:, k, sl],
                        start=(k == 0), stop=(k == KC - 1),
                    )
                qks = work.tile([128, 512], F32, tag="qks", bufs=3)
                nc.scalar.copy(qks[:], ps[:])
                rot = work.tile([128, 512], F32, tag="rot", bufs=3)
                for g in range(2):
                    o = g * 64
                    nc.sync.dma_start(rot[o:o + 32, :], qks[o + 32:o + 64, :])
                    nc.sync.dma_start(rot[o + 32:o + 64, :], qks[o:o + 32, :])
                tmp = work.tile([128, 512], F32, tag="tmp")
                nc.vector.tensor_mul(tmp[:], qks[:], cos_sb[:, sl])
                rots = work.tile([128, 512], F32, tag="rots")
                nc.vector.tensor_mul(rots[:], rot[:], sin_sb[:, sl])
                nc.vector.tensor_add(qkb[:, mb, sl], tmp[:], rots[:])

            # k0,k1 slab 0 then q0,q1 slab 0 (unblocks scores j=0-3),
            # then the remaining k slabs; all borrow the idle score banks
            emit_qk_tile(1, 0, "ps_s")
            emit_qk_tile(0, 0, "ps_s")
            for sb in range(1, 4):
                emit_qk_tile(1, sb, "ps_s")

            # head-2 partition-shift copies: qk2d = [k2 (lo) | q2 (hi)]
            qk2d = const.tile([128, S], BF16)

            # bias broadcast [128, 768] via K=1 matmuls (early, PE cheap)
            bp128 = const.tile([128, DIM], F32)
            for o0, on in ((0, 512), (512, 256)):
                psb = psum.tile([128, on], F32, tag="ps_mm")
                nc.tensor.matmul(
                    psb[:], ones128[0:1, :], bp_sb[0:1, o0:o0 + on],
                    start=True, stop=True,
                )
                nc.vector.tensor_copy(bp128[:, o0:o0 + on], psb[:])

            # v in [keys, ch]; slab per head = [v | ones | pad]; the memset-1
            # leaves pad columns at 1.0, which only writes harmless extra
            # denominator copies into unread psum rows 65-127
            v_aug = const.tile([128, NJ, HL * 128], BF16)
            nc.gpsimd.memset(v_aug[:], 1.0)

            def emit_v_chunk(st):
                ps = psum.tile([128, HL * DH], F32, tag="ps_mm")
                for k in range(KC):
                    nc.tensor.matmul(
                        ps[:],
                        xT_sb[:, k, st * 128:(st + 1) * 128],
                        wv_sb[:, k, :],
                        start=(k == 0), stop=(k == KC - 1),
                    )
                dst = v_aug[:, st, :].rearrange(
                    "p (h x) -> p h x", h=HL)[:, :, 0:DH]
                src = ps.rearrange("p (h x) -> p h x", h=HL)
                nc.vector.tensor_copy(dst, src)

            # ---- per-block predication flags -------------------------------
            with tc.tile_critical():
                conds = []
                for i in range(NB):
                    r = nc.gpsimd.alloc_register(f"flag_{i}")
                    nc.gpsimd.reg_load(r, flags[0:1, i:i + 1])
                    conds.append(nc.gpsimd.snap(r, donate=True, min_val=0,
                                                max_val=1))

            # ---- attention ------------------------------------------------
            # last DP[ib] head-2 chunk-pairs per block go to the DVE
            # poly-exp (PSUM-direct custom ops, HW-validated) to offload
            # the bottleneck ScalarE exp stream
            DP = [0, 3, 3, 2]

            def emit_dve_exp(ps2, dst):
                hh = work.tile([128, 2, 512], F32, tag="hh", bufs=2)
                hflat = hh[:].rearrange("p a b -> p (a b)")
                sflat = ps2[:].rearrange("p a b -> p (a b)")
                nc.vector._custom_dve(
                    EXP1, out=hflat, in0=sflat, in1=c4b[:],
                    s0=c7, s1=c6, imm2=c5,
                )
                nc.vector._custom_dve(
                    EXP2, out=dst, in0=sflat, in1=hflat,
                    s0=c3, s1=c2, imm2=c1,
                )

            agZ = []
            for b in range(NB):
                agZ_b = dram.tile([DIM, SC], BF16, tag=f"agZ{b}")
                agZ.append(agZ_b)
            P01 = const.tile([128, NJ, 2, 512], BF16)
            P2 = const.tile([128, NJ, 512], BF16)

            def emit_norm(ps_o, dst_d, r0, tag):
                # denominator sits on psum partition 64; copy to SBUF first
                # (custom-DVE reciprocal wants a plain SBUF operand), then
                # broadcast 1/den across partitions on idle GpSimd instead of
                # burning PE time on a K=1 matmul
                den = work.tile([1, 512], F32, tag="den")
                nc.vector.tensor_copy(den[:], ps_o[64:65, :])
                rcp = work.tile([1, 512], F32, tag="rcp")
                nc.vector.reciprocal_approx_fast(rcp[:], den[:])
                rcpb = work.tile([DH, 512], F32, tag="rcpb")
                nc.gpsimd.partition_broadcast(rcpb[:], rcp[:], channels=DH)
                onum = work.tile([DH, 512], F32, tag="onum")
                nc.vector.tensor_copy(onum[:], ps_o[0:DH, :])
                ob = work.tile([DH, 512], BF16, tag="ob", bufs=3)
                nc.vector.tensor_mul(ob[:], onum[:], rcpb[:])
                nc.sync.dma_start(dst_d[r0:r0 + DH, :], ob[:])

            for ib in range(NB):
                isl = slice(ib * 512, (ib + 1) * 512)
                ob012 = dram.tile([HL * DH, SC], BF16, tag=f"ob012_{ib}")

                # heads 0,1 on PE row-halves, same key chunk j
                for j in range(NJ):
                    ps2 = psum.tile([128, 2, 512], F32, tag="ps_s")
                    nc.tensor.matmul(
                        ps2[:, 0, :],
                        qkb[0:64, 1, j * 128:(j + 1) * 128],
                        qkb[0:64, 0, isl], start=True, stop=True,
                        tile_position=(0, 0),
                    )
                    nc.tensor.matmul(
                        ps2[:, 1, :],
                        qkb[64:128, 1, j * 128:(j + 1) * 128],
                        qkb[64:128, 0, isl], start=True, stop=True,
                        tile_position=(64, 0),
                    )
                    nc.scalar.activation(
                        P01[:, j, :, :], ps2[:], AF.Exp, scale=scale
                    )
                    if ib == 0:
                        # PE filler while ScalarE chews exp: v chunks,
                        # then q2k2 m-block + rope, then q01 slabs 1-3
                        if j < 8:
                            emit_v_chunk(2 * j)
                            emit_v_chunk(2 * j + 1)
                        elif j < 12:
                            emit_qk_tile(2, j - 8)
                        elif j < 15:
                            emit_qk_tile(0, j - 11)
                if ib == 0:
                    nc.sync.dma_start(qk2d[0:64, :], qkb[64:128, 2, :])
                    nc.sync.dma_start(qk2d[64:128, :], qkb[0:64, 2, :])

                for h in range(2):
                    ps_o = psum.tile([128, 512], F32, tag="ps_o")
                    for jc in range(NJ):
                        nc.tensor.matmul(
                            ps_o[:],
                            v_aug[:, jc, h * 128:(h + 1) * 128],
                            P01[:, jc, h, :],
                            start=(jc == 0), stop=(jc == NJ - 1),
                        )
                    emit_norm(ps_o, ob012, h * DH, f"{ib}_{h}")

                # head 2: even chunks on rows 0-63 (k2 copy), odd on 64-127
                for t in range(NJ // 2):
                    j0, j1 = 2 * t, 2 * t + 1
                    ps2 = psum.tile([128, 2, 512], F32, tag="ps_s")
                    nc.tensor.matmul(
                        ps2[:, 0, :],
                        qk2d[0:64, j0 * 128:(j0 + 1) * 128],
                        qkb[0:64, 2, isl], start=True, stop=True,
                        tile_position=(0, 0),
                    )
                    nc.tensor.matmul(
                        ps2[:, 1, :],
                        qkb[64:128, 2, j1 * 128:(j1 + 1) * 128],
                        qk2d[64:128, isl], start=True, stop=True,
                        tile_position=(64, 0),
                    )
                    if t < NJ // 2 - DP[ib]:
                        nc.scalar.activation(
                            P2[:, j0:j0 + 2, :], ps2[:], AF.Exp, scale=scale
                        )
                    else:
                        emit_dve_exp(ps2, P2[:, j0:j0 + 2, :])
                ps_o = psum.tile([128, 512], F32, tag="ps_o")
                for jc in range(NJ):
                    nc.tensor.matmul(
                        ps_o[:],
                        v_aug[:, jc, 2 * 128:3 * 128],
                        P2[:, jc, :],
                        start=(jc == 0), stop=(jc == NJ - 1),
                    )
                emit_norm(ps_o, ob012, 2 * DH, f"{ib}_2")

                nc.gpsimd.collective_compute(
                    "AllGather",
                    mybir.AluOpType.bypass,
                    replica_groups=GROUPS,
                    ins=[ob012.opt()],
                    outs=[agZ[ib][:]],
                )

            # keep the PE's HAM window busy while the last gather lands
            for w in range(12):
                wps = psum.tile([128, 512], F32, tag="ps_mm")
                nc.tensor.matmul(
                    wps[:], qkb[:, 0, 0:128], qkb[:, 1, 0:512],
                    start=True, stop=True,
                )

            # ---- output projection on my 512-row slice ---------------------
            # cond-predicated loads: only block g's flag is 1 on core g, so
            # each load depends only on its own block's gather
            ag_sb = const.tile([128, KC, SC], BF16)
            for b in range(NB):
                nc.gpsimd.dma_start(
                    ag_sb[:],
                    agZ[b][:].rearrange("(k p) n -> p k n", p=128),
                    cond=conds[b],
                )

            for m in range(SC // 128):
                for o0, on in ((0, 512), (512, 256)):
                    ps_p = psum.tile([128, on], F32, tag="ps_mm")
                    for k in range(KC):
                        nc.tensor.matmul(
                            ps_p[:],
                            ag_sb[:, k, m * 128:(m + 1) * 128],
                            wp_sb[:, k, o0:o0 + on],
                            start=(k == 0), stop=(k == KC - 1),
                        )
                    po = work.tile([128, on], F32, tag="po", bufs=4)
                    nc.vector.tensor_add(po[:], ps_p[:], bp128[:, o0:o0 + on])
                    nc.sync.dma_start(
                        out_d[m * 128:(m + 1) * 128, o0:o0 + on], po[:]
                    )

    nc.compile()
    return nc


def _rope_tables():
    bf16 = ml_dtypes.bfloat16
    inv = (1.0 / (THETA ** (np.arange(0, DH, 2, dtype=np.float32) / DH))).astype(
        np.float32
    )
    pos = np.arange(S, dtype=np.float32)
    f = pos[:, None] * inv[None, :]           # [S, 32] f32, matches reference
    c = np.cos(f).T.astype(np.float32)        # [32, S]
    s = np.sin(f).T.astype(np.float32)
    cos64 = np.concatenate([c, c], axis=0)    # rows i and 32+i = cos(f_i)
    sin64 = np.concatenate([-s, s], axis=0)   # sign folded for rotate_half
    return (
        np.concatenate([cos64, cos64], axis=0).astype(bf16),  # [128, S]
        np.concatenate([sin64, sin64], axis=0).astype(bf16),
    )


def _shard_inputs(x, W_qkv, W_proj, b_proj):
    bf16 = ml_dtypes.bfloat16
    cos128, sin128 = _rope_tables()
    # deinterleave perm: new[i] = orig[2i] (i<32), new[32+i] = orig[2i+1]
    perm = np.concatenate([np.arange(0, DH, 2), np.arange(1, DH, 2)])
    wp_t = np.ascontiguousarray(W_proj.T).astype(bf16)          # [c, o]
    bp_r = np.ascontiguousarray(b_proj[None, :]).astype(np.float32)
    in_maps = []
    for c in range(N_CORES):
        b, g = c // 4, c % 4
        hs = [HL * g + i for i in range(HL)]
        q_r = [h * DH + perm for h in hs]
        k_r = [DIM + h * DH + perm for h in hs]
        # column order [q0, q1 | k0, k1 | q2, k2] to align base partitions
        qk_rows = np.concatenate([q_r[0], q_r[1], k_r[0], k_r[1], q_r[2], k_r[2]])
        v_rows = np.concatenate([2 * DIM + h * DH + np.arange(DH) for h in hs])
        flag = np.zeros(NB, dtype=np.uint32)
        flag[g] = 1
        in_maps.append({
            "xT": np.ascontiguousarray(x[b].T).astype(bf16),
            "wqk": np.ascontiguousarray(W_qkv[qk_rows].T).astype(bf16),
            "wv": np.ascontiguousarray(W_qkv[v_rows].T).astype(bf16),
            "cosq": cos128,
            "sinq": sin128,
            "wp": wp_t,
            "bp": bp_r,
            "flags": flag[None, :],
        })
    return in_maps


def run(inputs, trace=False, tmpdir=None):
    if "nc" not in _CACHED:
        _CACHED["nc"] = _build()
    nc = _CACHED["nc"]
    in_maps = _shard_inputs(
        inputs["x"], inputs["W_qkv"], inputs["W_proj"], inputs["b_proj"]
    )
    res = bass_utils.run_bass_kernel_spmd(
        nc, in_maps, core_ids=list(range(N_CORES)), trace=trace, tmpdir=tmpdir
    )
    out = np.empty((B, S, DIM), dtype=np.float32)
    for c in range(N_CORES):
        b, g = c // 4, c % 4
        out[b, g * SC:(g + 1) * SC, :] = res.results[c]["out"]
    return out, res


def kernel(**inputs):
    out, _ = run(inputs, trace=False)
    return out

